# revision 67
# baseline (speedup 1.0000x reference)
"""MixProp GNN kernel for 8x Trainium2 NeuronCores.

Math (per batch b, X = x[b] as [N, (t,c)] node-major):
    A    = (adj + I) / deg[None, :]            (column-normalized, host)
    P1   = A @ X,  P2 = A @ P1                 (adjacency powers, on device)
    y    = sigmoid(V0 @ X + V1 @ P1 + V2 @ P2 + bias)
with the MixProp alpha-mixing folded into the projection weights:
    V0 = W0 + a*W1 + a*W2,  V1 = W1 + a*W2,  V2 = W2.

Speed strategy:
  * Propagation runs in fp8e4m3 with DoubleRow perf mode (two K-tiles per
    matmul pass, 2x PE throughput vs bf16). A is pre-scaled by SA=512 so its
    ~5e-4 entries land in e4m3's normal range; P1 is restaged to fp8 scaled
    to SP1=16 (copy scale SP1/SA applied during the PSUM->SBUF cast).
  * A is host-pre-blocked into the exact SBUF panel layout so every panel
    DMA moves 4 KiB/partition contiguously; the first NRES panels stay
    resident in SBUF across both steps.
  * P1's fp8 restage lives entirely in SBUF (no DRAM round trip).
  * The channel projection needs channel-major data. P1 spills node-major
    with large descriptors and returns through the hardware DMA transpose
    XBAR (dma_start(transpose=True)); P2 never touches DRAM -- its stage
    tiles are transposed on the PE (identity-matmul transpose) straight
    into a per-range SBUF slab. The projection then runs with t-expanded
    block-diagonal weights: lhsT[(t,c),(t',o)] = delta_tt' * V[o,c].
  * All 16-bit tensors are fp16 (smaller rounding than bf16, same speed);
    descale factors 1/SA, 1/(SA*SP1) are folded into the projection weights.

Sharding: data-parallel over batch B=8, one batch per core.
"""

import numpy as np

B, C, N, T = 8, 32, 4096, 32
ALPHA = 0.05
C_OUT = 32
CT = C * T            # 1024 free elems per node, flattened (t, c) order
NT = N * T
P = 128               # SBUF partitions
NW = N // P           # 32 contraction chunks
NV = N // P           # 32 output row tiles
FS = 512              # psum free-dim slice (one PSUM bank of fp32)
NF = CT // FS         # 2 free slices per row tile
SA = 512.0            # fp8 scale on A
SP1 = 16.0            # fp8 scale on restaged P1
NRES = 7              # A panels kept SBUF-resident across both steps
SPROJ = 1.0           # extra scale on the projection psum
TB = CT // P          # 8 (t,c) partition blocks
NRW = 1024            # node rows per XBAR transpose load
NR = N // NRW         # 4 row ranges per matrix
SPILL_ROWS = NRW      # rows per node-major spill dram tile


def _build_nc():
    import concourse.mybir as mybir
    from concourse import bacc
    from concourse.tile import TileContext

    F32 = mybir.dt.float32
    F16 = mybir.dt.float16
    F8 = mybir.dt.float8e4
    DR = mybir.MatmulPerfMode.DoubleRow

    nc = bacc.Bacc()

    a8_d = nc.dram_tensor("a8", [NV * P, N], F8, kind="ExternalInput")
    x8_d = nc.dram_tensor("x8", [P, NW * CT], F8, kind="ExternalInput")
    xt_d = nc.dram_tensor("xt", [CT, N], F16, kind="ExternalInput")
    vx_d = nc.dram_tensor("vx", [P, P], F16, kind="ExternalInput")
    v1_d = nc.dram_tensor("v1", [P, P], F16, kind="ExternalInput")
    v2_d = nc.dram_tensor("v2", [P, P], F16, kind="ExternalInput")
    bias_d = nc.dram_tensor("bias", [P, 1], F32, kind="ExternalInput")
    ident_d = nc.dram_tensor("ident", [P, P], F16, kind="ExternalInput")
    y_d = nc.dram_tensor("y", [CT, N], F16, kind="ExternalOutput")

    with TileContext(nc) as tc:
        with (
            tc.tile_pool(name="dram", bufs=1, space="DRAM") as dram_pool,
            tc.tile_pool(name="big", bufs=1) as big_pool,
            tc.tile_pool(name="panel", bufs=8) as panel_pool,
            tc.tile_pool(name="stage", bufs=4) as stage_pool,
            tc.tile_pool(name="slabx", bufs=8) as slabx_pool,
            tc.tile_pool(name="slab1", bufs=8) as slab1_pool,
            tc.tile_pool(name="slabp2", bufs=2) as slabp2_pool,
            tc.tile_pool(name="outp", bufs=6) as out_pool,
            tc.tile_pool(name="consts", bufs=1) as const_pool,
            tc.tile_pool(name="psum_p", bufs=3, space="PSUM") as psum_pool,
            tc.tile_pool(name="psum_y", bufs=3, space="PSUM") as psum_y_pool,
            tc.tile_pool(name="psum_t", bufs=2, space="PSUM") as psum_t_pool,
        ):
            # node-major spill targets, one dram tile per XBAR row-range so
            # the transpose loads only depend on the 8 spills they cover
            p1n_d = [
                dram_pool.tile([SPILL_ROWS, CT], F16, name=f"p1n{r}", tag=f"p1n{r}")
                for r in range(NR)
            ]

            x8_t = big_pool.tile([P, NW * CT], F8, tag="x8")
            # P1 restaged in fp8 (written by ACT from step-1 psum)
            p1rhs = big_pool.tile([P, NW * CT], F8, tag="p1rhs")
            # resident A panels
            a_res = big_pool.tile([P, NRES * N], F8, tag="ares")

            def issue_panel(vt, step, cache):
                # prefetched several iterations ahead so an XBAR wait at the
                # head of the SP queue can't starve the PE of panels
                if vt >= NV or vt in cache:
                    return
                if vt < NRES:
                    if step == 0:
                        nc.sync.dma_start(
                            a_res[:, vt * N:(vt + 1) * N],
                            a8_d[vt * P:(vt + 1) * P, :],
                        )
                    cache[vt] = a_res[:, vt * N:(vt + 1) * N]
                else:
                    panel = panel_pool.tile([P, N], F8, tag="panel")
                    nc.sync.dma_start(panel, a8_d[vt * P:(vt + 1) * P, :])
                    cache[vt] = panel

            slabs = {}
            p2slab = {}

            def prefetch_slabs(unit):
                # X slab load and P1 transpose never wait in step 2 (P1 was
                # spilled in step 1), so they can run well ahead of the unit
                nr, tb = divmod(unit, TB)
                xts = slabx_pool.tile([P, NRW], F16, tag="xts")
                nc.sync.dma_start(
                    xts, xt_d[tb * P:(tb + 1) * P, nr * NRW:(nr + 1) * NRW]
                )
                p1ts = slab1_pool.tile([P, NRW], F16, tag="p1ts")
                nc.sync.dma_start(
                    p1ts, p1n_d[nr][:, tb * P:(tb + 1) * P], transpose=True
                )
                slabs[unit] = (xts, p1ts)

            def project(unit):
                # one (tb, nr) projection unit: channel-mix X/P1/P2 via
                # t-expanded weights, sigmoid, write y rows
                nr, tb = divmod(unit, TB)
                xts, p1ts = slabs.pop(unit)
                p2ts = p2slab[nr][:, tb * NRW:(tb + 1) * NRW]
                # X and P1 terms for both halves first, the late-arriving
                # P2 terms last, so the PE has work while p2ts lands
                outp = out_pool.tile([P, NRW], F16, tag="outp")
                psys = []
                for h in range(NRW // FS):
                    psy = psum_y_pool.tile([P, FS], F32, tag="psy")
                    psys.append(psy)
                    lo, hi = h * FS, (h + 1) * FS
                    nc.tensor.matmul(
                        psy, vx_t, xts[:, lo:hi],
                        start=True, stop=False, skip_group_check=True,
                    )
                    nc.tensor.matmul(
                        psy, v1_t, p1ts[:, lo:hi],
                        start=False, stop=False, skip_group_check=True,
                    )
                for h in range(NRW // FS):
                    lo, hi = h * FS, (h + 1) * FS
                    nc.tensor.matmul(
                        psys[h], v2_t, p2ts[:, lo:hi],
                        start=False, stop=True, skip_group_check=True,
                    )
                    nc.scalar.activation(
                        outp[:, lo:hi],
                        psys[h],
                        mybir.ActivationFunctionType.Sigmoid,
                        bias=bias_t,
                    )
                nc.gpsimd.dma_start(
                    y_d[tb * P:(tb + 1) * P, nr * NRW:(nr + 1) * NRW], outp
                )

            def unit_slot(k):
                # projection unit k = (nr, tb) runs a few iterations after
                # the last spill (vt = 8*nr + 7) it depends on
                return 8 * (k // TB) + 11 + (k % TB)

            for step in range(2):
                rhs = x8_t if step == 0 else p1rhs
                cache = {}
                if step == 0:
                    # panel 0 first, then all of x8 (every row tile contracts
                    # the full x8, so it gates the whole first column)
                    XCH = NW * CT // 4
                    issue_panel(0, step, cache)
                    for i in range(4):
                        nc.sync.dma_start(
                            x8_t[:, i * XCH:(i + 1) * XCH],
                            x8_d[:, i * XCH:(i + 1) * XCH],
                        )
                for v in range(7):
                    issue_panel(v, step, cache)
                    vx_t = const_pool.tile([P, P], F16, tag="vx")
                    nc.sync.dma_start(vx_t, vx_d[:, :])
                    v1_t = const_pool.tile([P, P], F16, tag="v1")
                    nc.sync.dma_start(v1_t, v1_d[:, :])
                    v2_t = const_pool.tile([P, P], F16, tag="v2")
                    nc.sync.dma_start(v2_t, v2_d[:, :])
                    bias_t = const_pool.tile([P, 1], F32, tag="bias")
                    nc.sync.dma_start(bias_t, bias_d[:, :])
                    ident_t = const_pool.tile([P, P], F16, tag="ident")
                    nc.sync.dma_start(ident_t, ident_d[:, :])
                next_unit = 0
                next_slab = 0
                for vt in range(NV):
                    issue_panel(vt + 7, step, cache)
                    panel = cache[vt]
                    pview = panel.rearrange("p (w v) -> p w v", w=NW)
                    rview = rhs.rearrange("p (w f) -> p w f", w=NW)
                    stage = stage_pool.tile([P, CT], F16, tag="stage")
                    for fi in range(NF):
                        ps = psum_pool.tile([P, FS], F32, tag="ps")
                        for wc in range(NW // 2):
                            nc.tensor.matmul(
                                ps,
                                pview[:, 2 * wc:2 * wc + 2, :],
                                rview[
                                    :,
                                    2 * wc:2 * wc + 2,
                                    fi * FS:(fi + 1) * FS,
                                ],
                                start=(wc == 0),
                                stop=(wc == NW // 2 - 1),
                                perf_mode=DR,
                            )
                        if step == 0:
                            nc.scalar.activation(
                                p1rhs[:, vt * CT + fi * FS: vt * CT + (fi + 1) * FS],
                                ps,
                                mybir.ActivationFunctionType.Copy,
                                scale=SP1 / SA,
                            )
                        nc.vector.tensor_copy(stage[:, fi * FS:(fi + 1) * FS], ps)
                    if step == 0:
                        spill = p1n_d[vt // 8]
                        nc.gpsimd.dma_start(
                            spill[(vt % 8) * P:(vt % 8 + 1) * P, :], stage
                        )
                    else:
                        # P2 goes channel-major in SBUF via PE transposes:
                        # no DRAM round trip, no spill-wait coupling
                        nr = vt // 8
                        if vt % 8 == 0:
                            p2slab[nr] = slabp2_pool.tile(
                                [P, TB * NRW], F16,
                                name="p2slab", tag="p2slab",
                            )
                        sl = p2slab[nr].rearrange("p (tb n) -> p tb n", tb=TB)
                        for q in range(2):
                            psT = psum_t_pool.tile([P, FS], F16, tag="psT")
                            for j in range(4):
                                tb = 4 * q + j
                                nc.tensor.matmul(
                                    psT[:, j * P:(j + 1) * P],
                                    stage[:, tb * P:(tb + 1) * P],
                                    ident_t,
                                    is_transpose=True,
                                    skip_group_check=True,
                                )
                            nc.vector.tensor_copy(
                                sl[:, 4 * q:4 * q + 4,
                                   (vt % 8) * P:(vt % 8 + 1) * P],
                                psT.rearrange("p (j n) -> p j n", j=4),
                            )
                    if step == 1:
                        while (
                            next_slab < NR * TB
                            and unit_slot(next_slab) <= vt + 5
                        ):
                            prefetch_slabs(next_slab)
                            next_slab += 1
                        while next_unit < NR * TB and unit_slot(next_unit) <= vt:
                            project(next_unit)
                            next_unit += 1
                if step == 1:
                    for unit in range(next_unit, NR * TB):
                        while next_slab < min(unit + 8, NR * TB):
                            prefetch_slabs(next_slab)
                            next_slab += 1
                        project(unit)

    nc.compile()
    return nc


def kernel(x, adj, w, b):
    return _run(x, adj, w, b)[0]


def _run(x, adj, w, b, trace=False, trace_kwargs=None):
    import ml_dtypes
    from concourse.bass_utils import run_bass_kernel_spmd

    F8NP = ml_dtypes.float8_e4m3

    x = np.ascontiguousarray(x, dtype=np.float32)
    adj = np.asarray(adj, dtype=np.float32)
    w = np.asarray(w, dtype=np.float32)
    b = np.asarray(b, dtype=np.float32)

    # Column-normalized adjacency with self loops, pre-transposed, fp8-scaled,
    # and pre-blocked into the SBUF panel layout: a8[vt*P + p, wc*P + v] =
    # SA * A^T[wc*P + p, vt*P + v].
    adjp = adj + np.eye(N, dtype=np.float32)
    deg = adjp.sum(axis=1)
    at = (adjp.T / deg[:, None]) * SA
    a8 = np.ascontiguousarray(
        at.reshape(NW, P, NV, P).transpose(2, 1, 0, 3).reshape(NV * P, N)
    ).astype(F8NP)

    # Fold alpha-mixing + fp8 descales into t-expanded projection weights.
    w0, w1, w2 = w[:, 0:C], w[:, C:2 * C], w[:, 2 * C:3 * C]
    # all projection weights carry an extra SPROJ so the fp8 vx entries sit
    # in e4m3's normal range; the sigmoid's input scale divides it back out
    v0 = (w0 + ALPHA * w1 + ALPHA * w2) * SPROJ
    v1 = (w1 + ALPHA * w2) * (SPROJ / SA)
    v2 = w2 * (SPROJ / (SA * SP1))
    TS = T // TB  # 4 t values per partition block

    def texpand(v, dt):
        m = np.zeros((TS, C, TS, C_OUT), dtype=np.float32)
        for t in range(TS):
            m[t, :, t, :] = v.T
        return np.ascontiguousarray(m.reshape(P, P)).astype(dt)

    vx_h = texpand(v0, np.float16)
    v1_h = texpand(v1, np.float16)
    v2_h = texpand(v2, np.float16)
    bias_h = np.ascontiguousarray(np.tile(b, TS).reshape(P, 1), dtype=np.float32)
    ident_h = np.eye(P, dtype=np.float16)

    nc = _build_nc()

    in_maps = []
    for bi in range(B):
        xb = x[bi]                                     # [C, N, T]
        # node-major (t, c) free order, blocked [p, w, f]
        xn = xb.transpose(1, 2, 0).reshape(N, CT)      # [N, (t, c)]
        x8 = np.ascontiguousarray(
            xn.reshape(NW, P, CT).transpose(1, 0, 2).reshape(P, NW * CT)
        ).astype(F8NP)
        # channel-major X for the projection: [(t, c), N]
        xt = np.ascontiguousarray(
            xb.transpose(2, 0, 1).reshape(CT, N).astype(np.float16)
        )
        in_maps.append(
            {
                "a8": a8, "x8": x8, "xt": xt,
                "vx": vx_h, "v1": v1_h, "v2": v2_h, "bias": bias_h,
                "ident": ident_h,
            }
        )

    kwargs = dict(trace_kwargs or {})
    res = run_bass_kernel_spmd(
        nc, in_maps, core_ids=list(range(B)), trace=trace, **kwargs
    )
    # y rows are (tb, ts, o), cols n  ->  [o, n, t]
    ys = []
    for r in res.results:
        yt = np.asarray(r["y"], dtype=np.float32).reshape(TB, TS, C_OUT, N)
        ys.append(yt.transpose(2, 3, 0, 1).reshape(C_OUT, N, T))
    y = np.stack(ys, axis=0)
    return np.ascontiguousarray(y), res


# revision 75
# speedup vs baseline: 1.0716x; 1.0716x over previous
"""MixProp GNN kernel for 8x Trainium2 NeuronCores.

Math (per batch b, X = x[b] as [N, (t,c)] node-major):
    A    = (adj + I) / deg[None, :]            (column-normalized, host)
    P1   = A @ X,  P2 = A @ P1                 (adjacency powers, on device)
    y    = sigmoid(V0 @ X + V1 @ P1 + V2 @ P2 + bias)
with the MixProp alpha-mixing folded into the projection weights:
    V0 = W0 + a*W1 + a*W2,  V1 = W1 + a*W2,  V2 = W2.

Speed strategy:
  * Propagation runs in fp8e4m3 with DoubleRow perf mode (two K-tiles per
    matmul pass, 2x PE throughput vs bf16). A is pre-scaled by SA=512 so its
    ~5e-4 entries land in e4m3's normal range; P1 is restaged to fp8 scaled
    to SP1=16 (copy scale SP1/SA applied during the PSUM->SBUF cast).
  * A is host-pre-blocked into the exact SBUF panel layout so every panel
    DMA moves 4 KiB/partition contiguously; the first NRES panels stay
    resident in SBUF across both steps.
  * P1's fp8 restage lives entirely in SBUF (no DRAM round trip).
  * The channel projection needs channel-major data. P1 spills node-major
    with large descriptors and returns through the hardware DMA transpose
    XBAR (dma_start(transpose=True)); P2 never touches DRAM -- its stage
    tiles are transposed on the PE (identity-matmul transpose) straight
    into a per-range SBUF slab. The projection then runs with t-expanded
    block-diagonal weights: lhsT[(t,c),(t',o)] = delta_tt' * V[o,c].
  * All 16-bit tensors are fp16 (smaller rounding than bf16, same speed);
    descale factors 1/SA, 1/(SA*SP1) are folded into the projection weights.

Sharding: data-parallel over batch B=8, one batch per core.
"""

import numpy as np

B, C, N, T = 8, 32, 4096, 32
ALPHA = 0.05
C_OUT = 32
CT = C * T            # 1024 free elems per node, flattened (t, c) order
NT = N * T
P = 128               # SBUF partitions
NW = N // P           # 32 contraction chunks
NV = N // P           # 32 output row tiles
FS = 512              # psum free-dim slice (one PSUM bank of fp32)
NF = CT // FS         # 2 free slices per row tile
SA = 512.0            # fp8 scale on A
SP1 = 16.0            # fp8 scale on restaged P1
NRES = 7              # A panels kept SBUF-resident across both steps
SPROJ = 64.0          # extra scale on the projection psum (fp8 weights)
TB = CT // P          # 8 (t,c) partition blocks
NRW = 1024            # node rows per XBAR transpose load
NR = N // NRW         # 4 row ranges per matrix
SPILL_ROWS = NRW      # rows per node-major spill dram tile


def _build_nc():
    import concourse.mybir as mybir
    from concourse import bacc
    from concourse.tile import TileContext

    F32 = mybir.dt.float32
    F16 = mybir.dt.float16
    F8 = mybir.dt.float8e4
    DR = mybir.MatmulPerfMode.DoubleRow

    nc = bacc.Bacc()

    a8_d = nc.dram_tensor("a8", [NV * P, N], F8, kind="ExternalInput")
    x8_d = nc.dram_tensor("x8", [P, NW * CT], F8, kind="ExternalInput")
    xt_d = nc.dram_tensor("xt", [CT, N], F8, kind="ExternalInput")
    vp_d = nc.dram_tensor("vp", [P, 2 * P], F8, kind="ExternalInput")
    v2_d = nc.dram_tensor("v2", [P, P], F16, kind="ExternalInput")
    bias_d = nc.dram_tensor("bias", [P, 1], F32, kind="ExternalInput")
    ident_d = nc.dram_tensor("ident", [P, P], F16, kind="ExternalInput")
    ident8_d = nc.dram_tensor("ident8", [P, P], F8, kind="ExternalInput")
    y_d = nc.dram_tensor("y", [CT, N], F16, kind="ExternalOutput")

    with TileContext(nc) as tc:
        with (
            tc.tile_pool(name="dram", bufs=1, space="DRAM") as dram_pool,
            tc.tile_pool(name="big", bufs=1) as big_pool,
            tc.tile_pool(name="panel", bufs=8) as panel_pool,
            tc.tile_pool(name="stage", bufs=4) as stage_pool,
            tc.tile_pool(name="mega", bufs=2) as mega_pool,
            tc.tile_pool(name="slabp2", bufs=2) as slabp2_pool,
            tc.tile_pool(name="outp", bufs=6) as out_pool,
            tc.tile_pool(name="consts", bufs=1) as const_pool,
            tc.tile_pool(name="psum_p", bufs=2, space="PSUM") as psum_pool,
            tc.tile_pool(name="psum_y", bufs=3, space="PSUM") as psum_y_pool,
            tc.tile_pool(name="psum_t", bufs=3, space="PSUM") as psum_t_pool,
        ):
            x8_t = big_pool.tile([P, NW * CT], F8, tag="x8")
            # P1 restaged in fp8 (written by ACT from step-1 psum)
            p1rhs = big_pool.tile([P, NW * CT], F8, tag="p1rhs")
            # resident A panels
            a_res = big_pool.tile([P, NRES * N], F8, tag="ares")

            def issue_panel(vt, step, cache):
                # prefetched several iterations ahead so an XBAR wait at the
                # head of the SP queue can't starve the PE of panels
                if vt >= NV or vt in cache:
                    return
                if vt < NRES:
                    if step == 0:
                        nc.sync.dma_start(
                            a_res[:, vt * N:(vt + 1) * N],
                            a8_d[vt * P:(vt + 1) * P, :],
                        )
                    cache[vt] = a_res[:, vt * N:(vt + 1) * N]
                else:
                    panel = panel_pool.tile([P, N], F8, tag="panel")
                    nc.sync.dma_start(panel, a8_d[vt * P:(vt + 1) * P, :])
                    cache[vt] = panel

            mega = {}
            p2slab = {}

            def project(unit):
                # one (tb, nr) projection unit: X and P1 terms ride a single
                # fp8 DoubleRow matmul (the mega slab holds X and 16*P1
                # halves 8192 apart), P2 adds in fp16, then sigmoid
                nr, tb = divmod(unit, TB)
                mg = mega[nr].rearrange("p (i f) -> p i f", i=2)
                vp = vp_t.rearrange("p (i m) -> p i m", i=2)
                p2ts = p2slab[nr][:, tb * NRW:(tb + 1) * NRW]
                outp = out_pool.tile([P, NRW], F16, tag="outp")
                for h in range(NRW // FS):
                    psy = psum_y_pool.tile([P, FS], F32, tag="psy")
                    lo, hi = tb * NRW + h * FS, tb * NRW + (h + 1) * FS
                    nc.tensor.matmul(
                        psy, vp, mg[:, :, lo:hi],
                        start=True, stop=False, skip_group_check=True,
                        perf_mode=DR,
                    )
                    nc.tensor.matmul(
                        psy, v2_t, p2ts[:, h * FS:(h + 1) * FS],
                        start=False, stop=True, skip_group_check=True,
                    )
                    nc.scalar.activation(
                        outp[:, h * FS:(h + 1) * FS],
                        psy,
                        mybir.ActivationFunctionType.Sigmoid,
                        bias=bias_t,
                        scale=1.0 / SPROJ,
                    )
                nc.gpsimd.dma_start(
                    y_d[tb * P:(tb + 1) * P, nr * NRW:(nr + 1) * NRW], outp
                )

            def unit_slot(k):
                # projection unit k = (nr, tb) runs a few iterations after
                # the last spill (vt = 8*nr + 7) it depends on
                return 8 * (k // TB) + 8 + (k % TB)

            for step in range(2):
                rhs = x8_t if step == 0 else p1rhs
                cache = {}
                if step == 0:
                    # panel 0 first, then all of x8 (every row tile contracts
                    # the full x8, so it gates the whole first column)
                    XCH = NW * CT // 4
                    issue_panel(0, step, cache)
                    for i in range(4):
                        nc.sync.dma_start(
                            x8_t[:, i * XCH:(i + 1) * XCH],
                            x8_d[:, i * XCH:(i + 1) * XCH],
                        )
                for v in range(7):
                    issue_panel(v, step, cache)
                if step == 0:
                    vp_t = const_pool.tile([P, 2 * P], F8, tag="vp")
                    nc.sync.dma_start(vp_t, vp_d[:, :])
                    v2_t = const_pool.tile([P, P], F16, tag="v2")
                    nc.sync.dma_start(v2_t, v2_d[:, :])
                    bias_t = const_pool.tile([P, 1], F32, tag="bias")
                    nc.sync.dma_start(bias_t, bias_d[:, :])
                    ident_t = const_pool.tile([P, P], F16, tag="ident")
                    nc.sync.dma_start(ident_t, ident_d[:, :])
                    ident8_t = const_pool.tile([P, P], F8, tag="ident8")
                    nc.sync.dma_start(ident8_t, ident8_d[:, :])
                next_unit = 0
                for vt in range(NV):
                    issue_panel(vt + 7, step, cache)
                    panel = cache[vt]
                    pview = panel.rearrange("p (w v) -> p w v", w=NW)
                    rview = rhs.rearrange("p (w f) -> p w f", w=NW)
                    if step == 1:
                        stage = stage_pool.tile([P, CT], F16, tag="stage")
                    for fi in range(NF):
                        ps = psum_pool.tile([P, FS], F32, tag="ps")
                        for wc in range(NW // 2):
                            nc.tensor.matmul(
                                ps,
                                pview[:, 2 * wc:2 * wc + 2, :],
                                rview[
                                    :,
                                    2 * wc:2 * wc + 2,
                                    fi * FS:(fi + 1) * FS,
                                ],
                                start=(wc == 0),
                                stop=(wc == NW // 2 - 1),
                                perf_mode=DR,
                            )
                        if step == 0:
                            nc.scalar.activation(
                                p1rhs[:, vt * CT + fi * FS: vt * CT + (fi + 1) * FS],
                                ps,
                                mybir.ActivationFunctionType.Copy,
                                scale=SP1 / SA,
                            )
                        if step == 1:
                            nc.vector.tensor_copy(
                                stage[:, fi * FS:(fi + 1) * FS], ps
                            )
                    if step == 1:
                        # P2 (and P1, from its fp8 restage) go channel-major
                        # in SBUF via PE transposes: no DRAM round trips
                        nr = vt // 8
                        if vt % 8 == 0:
                            p2slab[nr] = slabp2_pool.tile(
                                [P, TB * NRW], F16,
                                name="p2slab", tag="p2slab",
                            )
                            mega[nr] = mega_pool.tile(
                                [P, 2 * TB * NRW], F8,
                                name="mega", tag="mega",
                            )
                        # one X slab section per iteration (contiguous rows)
                        tbx = vt % 8
                        nc.sync.dma_start(
                            mega[nr][:, tbx * NRW:(tbx + 1) * NRW],
                            xt_d[tbx * P:(tbx + 1) * P,
                                 nr * NRW:(nr + 1) * NRW],
                        )
                        sl = p2slab[nr].rearrange("p (tb n) -> p tb n", tb=TB)
                        sl1 = mega[nr][:, TB * NRW:].rearrange(
                            "p (tb n) -> p tb n", tb=TB
                        )
                        for q in range(2):
                            psT = psum_t_pool.tile([P, FS], F16, tag="psT")
                            for j in range(4):
                                tb = 4 * q + j
                                nc.tensor.matmul(
                                    psT[:, j * P:(j + 1) * P],
                                    stage[:, tb * P:(tb + 1) * P],
                                    ident_t,
                                    is_transpose=True,
                                    skip_group_check=True,
                                )
                            nc.vector.tensor_copy(
                                sl[:, 4 * q:4 * q + 4,
                                   (vt % 8) * P:(vt % 8 + 1) * P],
                                psT.rearrange("p (j n) -> p j n", j=4),
                            )
                        while next_unit < NR * TB and unit_slot(next_unit) <= vt:
                            project(next_unit)
                            next_unit += 1
                        for q in range(2):
                            # hw fp8 transpose writes with element step 2
                            psT1 = psum_t_pool.tile([P, 2 * FS], F8, tag="psT")
                            pv1 = psT1.rearrange(
                                "p (j n two) -> p two j n", two=2, j=4
                            )
                            for j in range(4):
                                tb = 4 * q + j
                                nc.tensor.matmul(
                                    pv1[:, 0, j, :],
                                    p1rhs[:, vt * CT + tb * P:
                                          vt * CT + (tb + 1) * P],
                                    ident8_t,
                                    is_transpose=True,
                                    skip_group_check=True,
                                )
                            nc.vector.tensor_copy(
                                sl1[:, 4 * q:4 * q + 4,
                                    (vt % 8) * P:(vt % 8 + 1) * P],
                                pv1[:, 0],
                            )
                if step == 1:
                    for unit in range(next_unit, NR * TB):
                        project(unit)

    nc.compile()
    return nc


def kernel(x, adj, w, b):
    return _run(x, adj, w, b)[0]


def _run(x, adj, w, b, trace=False, trace_kwargs=None):
    import ml_dtypes
    from concourse.bass_utils import run_bass_kernel_spmd

    F8NP = ml_dtypes.float8_e4m3

    x = np.ascontiguousarray(x, dtype=np.float32)
    adj = np.asarray(adj, dtype=np.float32)
    w = np.asarray(w, dtype=np.float32)
    b = np.asarray(b, dtype=np.float32)

    # Column-normalized adjacency with self loops, pre-transposed, fp8-scaled,
    # and pre-blocked into the SBUF panel layout: a8[vt*P + p, wc*P + v] =
    # SA * A^T[wc*P + p, vt*P + v].
    adjp = adj + np.eye(N, dtype=np.float32)
    deg = adjp.sum(axis=1)
    at = (adjp.T / deg[:, None]) * SA
    a8 = np.ascontiguousarray(
        at.reshape(NW, P, NV, P).transpose(2, 1, 0, 3).reshape(NV * P, N)
    ).astype(F8NP)

    # Fold alpha-mixing + fp8 descales into t-expanded projection weights.
    w0, w1, w2 = w[:, 0:C], w[:, C:2 * C], w[:, 2 * C:3 * C]
    # all projection weights carry an extra SPROJ so the fp8 vx entries sit
    # in e4m3's normal range; the sigmoid's input scale divides it back out
    v0 = (w0 + ALPHA * w1 + ALPHA * w2) * SPROJ
    v1 = (w1 + ALPHA * w2) * (SPROJ / SP1)
    v2 = w2 * (SPROJ / (SA * SP1))
    TS = T // TB  # 4 t values per partition block

    def texpand(v, dt):
        m = np.zeros((TS, C, TS, C_OUT), dtype=np.float32)
        for t in range(TS):
            m[t, :, t, :] = v.T
        return np.ascontiguousarray(m.reshape(P, P)).astype(dt)

    # X and P1 weights ride one fp8 DoubleRow stationary [vx | v1]
    vp_h = np.ascontiguousarray(
        np.concatenate(
            [texpand(v0, F8NP), texpand(v1, F8NP)], axis=1
        )
    )
    v2_h = texpand(v2, np.float16)
    bias_h = np.ascontiguousarray(np.tile(b, TS).reshape(P, 1), dtype=np.float32)
    ident_h = np.eye(P, dtype=np.float16)
    ident8_h = np.eye(P).astype(F8NP)

    nc = _build_nc()

    in_maps = []
    for bi in range(B):
        xb = x[bi]                                     # [C, N, T]
        # node-major (t, c) free order, blocked [p, w, f]
        xn = xb.transpose(1, 2, 0).reshape(N, CT)      # [N, (t, c)]
        x8 = np.ascontiguousarray(
            xn.reshape(NW, P, CT).transpose(1, 0, 2).reshape(P, NW * CT)
        ).astype(F8NP)
        # channel-major X for the projection: [(t, c), N]
        xt = np.ascontiguousarray(
            xb.transpose(2, 0, 1).reshape(CT, N).astype(F8NP)
        )
        in_maps.append(
            {
                "a8": a8, "x8": x8, "xt": xt,
                "vp": vp_h, "v2": v2_h, "bias": bias_h,
                "ident": ident_h, "ident8": ident8_h,
            }
        )

    kwargs = dict(trace_kwargs or {})
    res = run_bass_kernel_spmd(
        nc, in_maps, core_ids=list(range(B)), trace=trace, **kwargs
    )
    # y rows are (tb, ts, o), cols n  ->  [o, n, t]
    ys = []
    for r in res.results:
        yt = np.asarray(r["y"], dtype=np.float32).reshape(TB, TS, C_OUT, N)
        ys.append(yt.transpose(2, 3, 0, 1).reshape(C_OUT, N, T))
    y = np.stack(ys, axis=0)
    return np.ascontiguousarray(y), res
